# revision 1
# baseline (speedup 1.0000x reference)
"""Trainium2 Bass kernel for additive-attention pooling.

Reference math (per sample b):
    score  = tanh(x @ W_w + W_b)          # [T, U]
    logits = score @ V_w + V_b            # [T, 1]
    attn   = softmax(logits, axis=T)
    out    = sum_t attn[t] * x[t, :]      # [D]

Shapes: x [64, 4096, 256] f32, W_w [256, 256], W_b [256], V_w [256, 1], V_b [1].
V_b shifts every logit of a sample equally, so it cancels in the softmax.

Strategy: data-parallel over batch, 8 samples per core on 8 NeuronCores.
All heavy math runs on TensorE in bf16:
  - GEMM1 computes score TRANSPOSED ([u, t] layout): lhsT = W block, rhs = xT.
    The host ships x pre-transposed (bf16) so no on-chip transpose is needed.
  - tanh on ScalarE (PSUM -> SBUF, per-partition bias = W_b chunk).
  - V-dot as tiny matmuls: lhsT = tanh-score [u, t-chunk], rhs = V [u, 1],
    giving logits in [t, 1] layout -- exactly the layout the weighted sum
    needs for its stationary operand.
  - exp on ScalarE ([128, 4] batches).
  - weighted sum: lhsT = w [t, 1], rhs = x natural (host also ships x in
    natural layout, bf16, with a ones column appended so the softmax
    denominator falls out of the same matmul).
Softmax max-subtraction is skipped: |logit| <= sum|V| < 20, safely in fp32/bf16
exp range.
"""

import numpy as np
import ml_dtypes

# ---- problem constants (hardcoded; kernel.py must be self-contained) ----
B, T, D, U = 64, 4096, 256, 256
N_CORES = 8
S = B // N_CORES          # samples per core
TT = 512                  # t-tile (rows per pipeline step)
N_TILES = T // TT         # tiles per sample
CH = TT // 128            # 128-row chunks per tile

BF16 = ml_dtypes.bfloat16

_CACHE = {}


def _build():
    import concourse.bass as bass
    import concourse.tile as tile
    from concourse import bacc, mybir
    from concourse.bass import ds, ts

    f32 = mybir.dt.float32
    bf16 = mybir.dt.bfloat16
    Tanh = mybir.ActivationFunctionType.Tanh
    Exp = mybir.ActivationFunctionType.Exp

    nc = bacc.Bacc("TRN2", target_bir_lowering=False, debug=False)

    xT_d = nc.dram_tensor("xT", [S, D, T], bf16, kind="ExternalInput").ap()
    xn_d = nc.dram_tensor("xn", [S, T, D + 1], bf16, kind="ExternalInput").ap()
    w_d = nc.dram_tensor("w", [D, U], bf16, kind="ExternalInput").ap()
    wb_d = nc.dram_tensor("wb", [128, U // 128], f32, kind="ExternalInput").ap()
    v_d = nc.dram_tensor("v", [128, U // 128], bf16, kind="ExternalInput").ap()
    out_d = nc.dram_tensor("out", [S, D], f32, kind="ExternalOutput").ap()

    NG = S * N_TILES  # total pipeline steps (64)

    with tile.TileContext(nc) as tc:
        with (
            tc.tile_pool(name="const", bufs=1) as const_pool,
            tc.tile_pool(name="xT", bufs=4) as xT_pool,
            tc.tile_pool(name="xn", bufs=6) as xn_pool,
            tc.tile_pool(name="tanh", bufs=3) as tanh_pool,
            tc.tile_pool(name="wexp", bufs=3) as wexp_pool,
            tc.tile_pool(name="fin", bufs=2) as fin_pool,
            tc.tile_pool(name="score_ps", bufs=3, space="PSUM") as score_pool,
            tc.tile_pool(name="logit_ps", bufs=3, space="PSUM") as logit_pool,
            tc.tile_pool(name="c_ps", bufs=2, space="PSUM") as c_pool,
        ):
            # constants
            w_sb = const_pool.tile([128, 2, U], bf16)     # [d_in_chunk, d_chunk, u]
            nc.sync.dma_start(w_sb[:], w_d.rearrange("(k p) u -> p k u", p=128))
            v_sb = const_pool.tile([128, 2], bf16)        # [u_in_chunk, u_chunk]
            nc.sync.dma_start(v_sb[:], v_d)
            wb_sb = const_pool.tile([128, 2], f32)
            nc.sync.dma_start(wb_sb[:], wb_d)

            tanh_tiles = {}
            xn_tiles = {}
            wexp_tiles = {}
            logit_tiles = {}
            c_ps = None

            for g in range(NG + 2):
                # ---- front: DMA + GEMM1 + tanh for step g ----
                if g < NG:
                    s, tt = divmod(g, N_TILES)
                    xT_t = xT_pool.tile([128, 2, TT], bf16)
                    nc.sync.dma_start(
                        xT_t[:],
                        xT_d[s, :, ts(tt, TT)].rearrange("(k p) t -> p k t", p=128),
                    )
                    xn_t = xn_pool.tile([128, CH, D + 1], bf16)
                    nc.sync.dma_start(
                        xn_t[:],
                        xn_d[s, ts(tt, TT), :].rearrange("(c p) f -> p c f", p=128),
                    )
                    xn_tiles[g] = xn_t

                    scores = []
                    for uc in range(2):
                        sc = score_pool.tile([128, TT], f32, tag="score")
                        for dc in range(2):
                            nc.tensor.matmul(
                                sc[:],
                                w_sb[:, dc, ts(uc, 128)],
                                xT_t[:, dc, :],
                                start=(dc == 0),
                                stop=(dc == 1),
                            )
                        scores.append(sc)

                # ---- exp + weighted sum for step g-2 (before tanh(g) on ACT) ----
                if 0 <= g - 2 < NG:
                    j = g - 2
                    sj, ttj = divmod(j, N_TILES)
                    lg = logit_tiles.pop(j)
                    wx = wexp_pool.tile([128, CH], bf16)
                    nc.scalar.activation(wx[:], lg[:], Exp)
                    wexp_tiles[j] = wx

                # ---- tanh for step g ----
                if g < NG:
                    tanh_t = tanh_pool.tile([128, 2, TT], bf16)
                    for uc in range(2):
                        nc.scalar.activation(
                            tanh_t[:, uc, :],
                            scores[uc][:],
                            Tanh,
                            bias=wb_sb[:, ds(uc, 1)],
                        )
                    tanh_tiles[g] = tanh_t

                # ---- V-dot (logits) for step g-1 ----
                if 0 <= g - 1 < NG:
                    j = g - 1
                    th = tanh_tiles.pop(j)
                    lg = logit_pool.tile([128, CH], f32, tag="logit")
                    for c in range(CH):
                        for uc in range(2):
                            nc.tensor.matmul(
                                lg[:, ds(c, 1)],
                                th[:, uc, ts(c, 128)],
                                v_sb[:, ds(uc, 1)],
                                start=(uc == 0),
                                stop=(uc == 1),
                            )
                    logit_tiles[j] = lg

                # ---- weighted-sum matmuls for step g-2 ----
                if 0 <= g - 2 < NG:
                    j = g - 2
                    sj, ttj = divmod(j, N_TILES)
                    wx = wexp_tiles.pop(j)
                    xn_t = xn_tiles.pop(j)
                    if ttj == 0:
                        c_ps = c_pool.tile([1, D + 1], f32, tag="acc")
                    for c in range(CH):
                        nc.tensor.matmul(
                            c_ps[:],
                            wx[:, ds(c, 1)],
                            xn_t[:, c, :],
                            start=(ttj == 0 and c == 0),
                            stop=(ttj == N_TILES - 1 and c == CH - 1),
                        )
                    if ttj == N_TILES - 1:
                        # context = c[0:D] / c[D]
                        recip = fin_pool.tile([1, 1], f32, tag="recip")
                        nc.vector.reciprocal(recip[:], c_ps[0:1, D : D + 1])
                        row = fin_pool.tile([1, D], f32, tag="row")
                        nc.vector.tensor_scalar_mul(
                            row[:], c_ps[0:1, 0:D], recip[:]
                        )
                        nc.sync.dma_start(out_d[ds(sj, 1), :], row[:])

    nc.compile()
    return nc


def _prep_inputs(inputs, W_w, W_b, V_w, V_b):
    x = np.asarray(inputs, dtype=np.float32)
    ones = np.ones((B, T, 1), dtype=np.float32)
    xn_full = np.concatenate([x, ones], axis=2).astype(BF16)      # [B, T, D+1]
    xT_full = np.ascontiguousarray(x.transpose(0, 2, 1)).astype(BF16)  # [B, D, T]

    w = np.asarray(W_w, dtype=np.float32).astype(BF16)            # [D, U]
    wb = np.asarray(W_b, dtype=np.float32).reshape(U // 128, 128).T.copy()  # [128, 2]
    v = (
        np.asarray(V_w, dtype=np.float32)
        .reshape(U // 128, 128)
        .T.copy()
        .astype(BF16)
    )  # [128, 2]

    in_maps = []
    for c in range(N_CORES):
        sl = slice(c * S, (c + 1) * S)
        in_maps.append(
            {
                "xT": np.ascontiguousarray(xT_full[sl]),
                "xn": np.ascontiguousarray(xn_full[sl]),
                "w": w,
                "wb": wb,
                "v": v,
            }
        )
    return in_maps


def kernel(inputs, W_w, W_b, V_w, V_b):
    from concourse.bass_utils import run_bass_kernel_spmd

    if "nc" not in _CACHE:
        _CACHE["nc"] = _build()
    nc = _CACHE["nc"]

    in_maps = _prep_inputs(inputs, W_w, W_b, V_w, V_b)
    res = run_bass_kernel_spmd(nc, in_maps, core_ids=list(range(N_CORES)))
    out = np.concatenate([r["out"] for r in res.results], axis=0)
    return np.asarray(out, dtype=np.float32)


# revision 4
# speedup vs baseline: 1.0329x; 1.0329x over previous
"""Trainium2 Bass kernel for additive-attention pooling.

Reference math (per sample b):
    score  = tanh(x @ W_w + W_b)          # [T, U]
    logits = score @ V_w + V_b            # [T, 1]
    attn   = softmax(logits, axis=T)
    out    = sum_t attn[t] * x[t, :]      # [D]

Shapes: x [64, 4096, 256] f32, W_w [256, 256], W_b [256], V_w [256, 1], V_b [1].
V_b shifts every logit of a sample equally, so it cancels in the softmax.

Strategy: data-parallel over batch, 8 samples per core on 8 NeuronCores.
All heavy math runs on TensorE in bf16:
  - GEMM1 computes score TRANSPOSED ([u, t] layout): lhsT = W block, rhs = xT.
    The host ships x pre-transposed (bf16) so no on-chip transpose is needed.
  - tanh on ScalarE (PSUM -> SBUF, per-partition bias = W_b chunk).
  - V-dot as N=1 matmuls: lhsT = tanh-score [u, t-chunk], rhs = V [u, 1],
    giving logits in [t, 1] layout -- exactly the layout the weighted sum
    needs for its stationary operand. These tiny matmuls are interleaved
    between the long GEMM/weighted-sum matmuls so their LDWEIGHTS hide.
  - exp on ScalarE, batched once per sample ([128, 32]).
  - weighted sum: lhsT = w [t, 1], rhs = x natural (host also ships x in
    natural layout, bf16, with a ones column appended so the softmax
    denominator falls out of the same matmul). Lags one sample behind the
    GEMM pipeline so it never waits on exp.
Softmax max-subtraction is skipped: |logit| <= sum|V| < 20, safely in fp32/bf16
exp range.
"""

import numpy as np
import ml_dtypes

# ---- problem constants (hardcoded; kernel.py must be self-contained) ----
B, T, D, U = 64, 4096, 256, 256
N_CORES = 8
S = B // N_CORES          # samples per core
TT = 512                  # t-tile (rows per pipeline step)
N_TILES = T // TT         # tiles per sample (8)
CH = TT // 128            # 128-row chunks per tile (4)
LAG_W = N_TILES + 1       # weighted-sum lag in tiles (one sample + 1)

BF16 = ml_dtypes.bfloat16

_CACHE = {}


def _build():
    import concourse.bass as bass
    import concourse.tile as tile
    from concourse import bacc, mybir
    from concourse.bass import ds, ts

    f32 = mybir.dt.float32
    bf16 = mybir.dt.bfloat16
    Tanh = mybir.ActivationFunctionType.Tanh
    Exp = mybir.ActivationFunctionType.Exp

    nc = bacc.Bacc("TRN2", target_bir_lowering=False, debug=False)

    xT_d = nc.dram_tensor("xT", [S, D, T], bf16, kind="ExternalInput").ap()
    xn_d = nc.dram_tensor("xn", [S, T, D + 1], bf16, kind="ExternalInput").ap()
    w_d = nc.dram_tensor("w", [D, U], bf16, kind="ExternalInput").ap()
    wb_d = nc.dram_tensor("wb", [128, U // 128], f32, kind="ExternalInput").ap()
    v_d = nc.dram_tensor("v", [128, U // 128], bf16, kind="ExternalInput").ap()
    out_d = nc.dram_tensor("out", [S, D], f32, kind="ExternalOutput").ap()

    NG = S * N_TILES  # total pipeline steps (64)

    with tile.TileContext(nc) as tc:
        with (
            tc.tile_pool(name="const", bufs=1) as const_pool,
            tc.tile_pool(name="xT", bufs=4) as xT_pool,
            tc.tile_pool(name="xn", bufs=LAG_W + 3) as xn_pool,
            tc.tile_pool(name="tanh", bufs=3) as tanh_pool,
            tc.tile_pool(name="wexp", bufs=2) as wexp_pool,
            tc.tile_pool(name="fin", bufs=2) as fin_pool,
            tc.tile_pool(name="score_ps", bufs=4, space="PSUM") as score_pool,
            tc.tile_pool(name="logit_ps", bufs=2, space="PSUM") as logit_pool,
            tc.tile_pool(name="c_ps", bufs=2, space="PSUM") as c_pool,
        ):
            # constants
            w_sb = const_pool.tile([128, 2, U], bf16)     # [d_in_chunk, d_chunk, u]
            nc.sync.dma_start(w_sb[:], w_d.rearrange("(k p) u -> p k u", p=128))
            v_sb = const_pool.tile([128, 2], bf16)        # [u_in_chunk, u_chunk]
            nc.sync.dma_start(v_sb[:], v_d)
            wb_sb = const_pool.tile([128, 2], f32)
            nc.sync.dma_start(wb_sb[:], wb_d)

            tanh_tiles = {}     # g -> tanh tile
            xn_tiles = {}       # g -> xn tile
            logit_tiles = {}    # sample -> [128, N_TILES*CH] psum tile
            wexp_tiles = {}     # sample -> [128, N_TILES*CH] bf16 weights
            c_tiles = {}        # sample -> [1, D+1] psum accumulator

            def emit_l2_pair(pair):
                """One chunk-column of the V-dot for tile j (closed group)."""
                j, c = pair
                sj, ttj = divmod(j, N_TILES)
                th = tanh_tiles[j]
                lg = logit_tiles[sj]
                for uc in range(2):
                    nc.tensor.matmul(
                        lg[:, ds(ttj * CH + c, 1)],
                        th[:, uc, ts(c, 128)],
                        v_sb[:, ds(uc, 1)],
                        start=(uc == 0),
                        stop=(uc == 1),
                    )
                if c == CH - 1:
                    del tanh_tiles[j]

            def emit_wsum_chunk(j, c):
                """One 128-row chunk of the weighted sum for tile j."""
                sj, ttj = divmod(j, N_TILES)
                wx = wexp_tiles[sj]
                xn_t = xn_tiles[j]
                nc.tensor.matmul(
                    c_tiles[sj][:],
                    wx[:, ds(ttj * CH + c, 1)],
                    xn_t[:, c, :],
                    start=(ttj == 0 and c == 0),
                    stop=(ttj == N_TILES - 1 and c == CH - 1),
                )
                if c == CH - 1:
                    del xn_tiles[j]

            for g in range(NG + LAG_W + 1):
                s, tt = divmod(g, N_TILES) if g < NG else (None, None)
                jw = g - LAG_W  # tile index for weighted sum this iteration
                jl = g - 1      # tile index for V-dot this iteration

                # L2 pairs for tile jl: u0 chunks first (tanh_u0 finishes
                # earlier), then u1 chunks.
                l2_pairs = []
                if 0 <= jl < NG:
                    l2_pairs = [(jl, c) for c in range(CH)]

                # ---- DMA for step g ----
                if g < NG:
                    xT_t = xT_pool.tile([128, 2, TT], bf16)
                    nc.sync.dma_start(
                        xT_t[:],
                        xT_d[s, :, ts(tt, TT)].rearrange("(k p) t -> p k t", p=128),
                    )
                    xn_t = xn_pool.tile([128, CH, D + 1], bf16)
                    nc.sync.dma_start(
                        xn_t[:],
                        xn_d[s, ts(tt, TT), :].rearrange("(c p) f -> p c f", p=128),
                    )
                    xn_tiles[g] = xn_t
                    if tt == 0:
                        logit_tiles[s] = logit_pool.tile(
                            [128, N_TILES * CH], f32, tag="logit", name=f"logit{s}"
                        )
                    if tt == 0:
                        c_tiles[s] = c_pool.tile([1, D + 1], f32, tag="acc", name=f"acc{s}")

                # ---- PE stream: GEMM + wsum with L2 pairs interleaved ----
                scores = []
                li = 0
                if g < NG:
                    for uc in range(2):
                        sc = score_pool.tile([128, TT], f32, tag="score")
                        for dc in range(2):
                            nc.tensor.matmul(
                                sc[:],
                                w_sb[:, dc, ts(uc, 128)],
                                xT_t[:, dc, :],
                                start=(dc == 0),
                                stop=(dc == 1),
                            )
                        if li < len(l2_pairs):
                            emit_l2_pair(l2_pairs[li])
                            li += 1
                        scores.append(sc)
                if 0 <= jw < NG:
                    for c in range(CH):
                        emit_wsum_chunk(jw, c)
                        if li < len(l2_pairs):
                            emit_l2_pair(l2_pairs[li])
                            li += 1
                while li < len(l2_pairs):
                    emit_l2_pair(l2_pairs[li])
                    li += 1

                # ---- ACT: tanh for step g ----
                if g < NG:
                    tanh_t = tanh_pool.tile([128, 2, TT], bf16)
                    for uc in range(2):
                        nc.scalar.activation(
                            tanh_t[:, uc, :],
                            scores[uc][:],
                            Tanh,
                            bias=wb_sb[:, ds(uc, 1)],
                        )
                    tanh_tiles[g] = tanh_t

                # ---- ACT: exp once per sample (after last tile's V-dot) ----
                if 0 <= jl < NG and jl % N_TILES == N_TILES - 1:
                    sj = jl // N_TILES
                    lg = logit_tiles.pop(sj)
                    wx = wexp_pool.tile([128, N_TILES * CH], bf16, tag="wexp")
                    nc.scalar.activation(wx[:], lg[:], Exp)
                    wexp_tiles[sj] = wx

                # ---- finalize sample after its last wsum chunk ----
                if 0 <= jw < NG and jw % N_TILES == N_TILES - 1:
                    sj = jw // N_TILES
                    del wexp_tiles[sj]
                    c_ps = c_tiles.pop(sj)
                    recip = fin_pool.tile([1, 1], f32, tag="recip")
                    nc.vector.reciprocal(recip[:], c_ps[0:1, D : D + 1])
                    row = fin_pool.tile([1, D], f32, tag="row")
                    nc.vector.tensor_scalar_mul(row[:], c_ps[0:1, 0:D], recip[:])
                    nc.sync.dma_start(out_d[ds(sj, 1), :], row[:])

    nc.compile()
    return nc


def _prep_inputs(inputs, W_w, W_b, V_w, V_b):
    x = np.asarray(inputs, dtype=np.float32)
    ones = np.ones((B, T, 1), dtype=np.float32)
    xn_full = np.concatenate([x, ones], axis=2).astype(BF16)      # [B, T, D+1]
    xT_full = np.ascontiguousarray(x.transpose(0, 2, 1)).astype(BF16)  # [B, D, T]

    w = np.asarray(W_w, dtype=np.float32).astype(BF16)            # [D, U]
    wb = np.asarray(W_b, dtype=np.float32).reshape(U // 128, 128).T.copy()  # [128, 2]
    v = (
        np.asarray(V_w, dtype=np.float32)
        .reshape(U // 128, 128)
        .T.copy()
        .astype(BF16)
    )  # [128, 2]

    in_maps = []
    for c in range(N_CORES):
        sl = slice(c * S, (c + 1) * S)
        in_maps.append(
            {
                "xT": np.ascontiguousarray(xT_full[sl]),
                "xn": np.ascontiguousarray(xn_full[sl]),
                "w": w,
                "wb": wb,
                "v": v,
            }
        )
    return in_maps


def kernel(inputs, W_w, W_b, V_w, V_b):
    from concourse.bass_utils import run_bass_kernel_spmd

    if "nc" not in _CACHE:
        _CACHE["nc"] = _build()
    nc = _CACHE["nc"]

    in_maps = _prep_inputs(inputs, W_w, W_b, V_w, V_b)
    res = run_bass_kernel_spmd(nc, in_maps, core_ids=list(range(N_CORES)))
    out = np.concatenate([r["out"] for r in res.results], axis=0)
    return np.asarray(out, dtype=np.float32)
